# revision 60
# baseline (speedup 1.0000x reference)
"""Bidirectional temporal attention on 8 Trainium2 NeuronCores.

Problem: x[1,16,256,768] -> per-head QKV projection (12 heads, hd=64),
heads 0-5 causal ("lookback"), heads 6-11 anti-causal ("lookahead"),
softmax over keys, concat heads, output projection.

Sharding: queries are strided-interleaved across the 8 cores
(core c owns queries q with q % 8 == c).  This makes the program
SPMD-uniform: every core runs the identical instruction stream; all
core-dependence lives in the input data (its x columns and its mask
tables).  K/V are computed sharded (core c projects sequence rows
[512c, 512c+512)) and shared with two AllGathers (K right after K-proj
so score matmuls unblock early; V follows).  Bounce buffers use the
SBUF-order layout so stage-out and unpack DMAs are contiguous 6KB runs
per partition (128 descriptors instead of 768).

On-chip layout: scores are computed transposed (S^T[k, q]) so the
softmax denominator comes for free from a ones-column appended to V
(PV matmul accumulates sum(exp) in row 64).  Score matmuls pack the
two heads of a pair into PE row groups 0-63 / 64-127 (hd=64
contraction) and are narrowed to each group's causal-staircase
boundary; PV matmuls are narrowed per k-tile to the staircase (the
sub-boundary part of pt is never written or read).

exp(): split across engines to balance the elementwise pipeline.
Diagonal-strip score tiles go through a custom DVE op that FUSES the
causal mask multiply into the poly-exp pass (Src1 = mask table, zero
extra instructions); off-diagonal tiles run ACT's table exp, with a
few groups routed to a plain DVE poly-exp for load balance.  The b3
block (all-diagonal) splits one head to ACT + explicit DVE mask-mul
since ACT has no off-diagonal work there.  exp() uses no
max-subtraction: |score| <= ~2 for this problem (verified in test.py);
the poly exp (deg-2 minimax ^8, Horner) has <=0.6% max rel err.

Normalization: pv PSUM is copied to SBUF on ACT immediately (frees the
PSUM bank so the next head-pair's PV starts without waiting on the
reciprocal chain); reciprocal on DVE, partition-broadcast and the
normalize multiply on the otherwise-idle GPSIMD.

fp8 was evaluated and rejected: quantizing V to e4m3 alone produces
2.8e-2 max rel err (tolerance 2e-2); DoubleRow needs both operands
fp8, so the 2x PV speedup is unreachable at this tolerance.
"""
import os
import sys

sys.path.insert(0, "/opt/trn_rl_repo")

import numpy as np
import ml_dtypes

import concourse.bass as bass
import concourse.bacc as bacc
import concourse.tile as tile
from concourse import mybir
from concourse.bass_utils import run_bass_kernel_spmd

BF16 = ml_dtypes.bfloat16

S = 4096          # sequence length (16*256)
D = 768           # model dim
H = 12            # heads
HD = 64           # head dim
NLB = 6           # lookback heads
NC = 8            # cores
QC = S // NC      # queries per core (512)
CH = D // 128     # contraction chunks (6)
KT_N = S // 128   # k-tiles (32)
SCALE = 1.0 / 8.0 # 1/sqrt(hd)

_BUILT = None
LAST_RESULT = None

# --- custom DVE exp: out = (c0 + c1*s + c2*s^2)^8 ~= exp(s/8) -------------
# deg-2 minimax then 3 squarings; max rel err 0.52% over |s|<=16 (scores
# here have |s| <= 15.6).
_EXPC = (1.00011951, 0.0157464011, 0.000121594115)
# 2-coef variant with c0 pinned to 1 (the masked op has no imm2 slot):
# (1 + c1*s + c2*s^2)^8, max rel err 0.57% over |s|<=16.
_EXPC2 = (0.01574234, 0.000122)
_EXP4_OP = None


def _register_exp_op():
    """Register two custom DVE ops:
    EXP8_ANT  : poly exp(s/8)               (off-diagonal tiles)
    EXP8M_ANT : poly exp(s/8) * Src1 (mask) (diagonal tiles, fused mask)
    Returns (exp_op, expm_op)."""
    global _EXP4_OP
    if _EXP4_OP is not None:
        return _EXP4_OP
    from concourse import dve_ops
    from concourse.dve_spec import Spec, Src0, Src1, C0, C1, C2, sq, lower
    from concourse.dve_uop import DveOpSpec

    from concourse.dve_spec import One

    def reg(name, body, ref, rd1):
        spec = Spec(body=body, reference=ref)
        row = max(dve_ops._SUB_OPCODE_FOR_NAME.values()) + 1
        dve_ops._SUB_OPCODE_FOR_NAME[name] = row
        shas = {}
        for ver in ("v3", "v4"):
            uops = lower(spec, ver=ver)
            shas[ver] = DveOpSpec(name=name, opcode=row, uops=uops,
                                  rd1_en=rd1).sha(ver)
        op = dve_ops.DveOp(name, spec, subdim=False, uops_sha=shas)
        dve_ops.OPS.append(op)
        dve_ops.CUSTOM_DVE_SPECS[name] = spec
        return op

    op_plain = reg(
        "EXP8_ANT",
        sq(sq(sq(C0 + Src0 * (C1 + Src0 * C2)))),
        lambda in0, in1, c0, c1, c2: (c0 + c1 * in0 + c2 * in0 * in0) ** 8,
        rd1=False)
    # masked variant: no imm2 slot available with a 2D in1, so c0 is
    # pinned to 1 and (c1, c2) ride in s0/s1 (_EXPC2).
    op_mask = reg(
        "EXP8M_ANT",
        sq(sq(sq(One + Src0 * (C0 + Src0 * C1)))) * Src1,
        lambda in0, in1, c0, c1, c2:
            ((1.0 + c0 * in0 + c1 * in0 * in0) ** 8) * in1,
        rd1=True)
    _EXP4_OP = (op_plain, op_mask)
    return _EXP4_OP





# Persistent NEFF cache: compile_bir_kernel is content-pure (BIR json ->
# neff bytes), so cache across processes/directories keyed by sha256.
_NEFF_CACHE_DIR = os.path.expanduser("~/.cache/bass_neff_cache")


def _install_neff_cache():
    import hashlib
    import shutil
    from concourse import bass_utils, bass2jax

    if getattr(bass_utils.compile_bir_kernel, "_cached_wrapper", False):
        return
    orig = bass_utils.compile_bir_kernel

    def cached(bir_json, tmpdir, neff_name="file.neff"):
        try:
            os.makedirs(_NEFF_CACHE_DIR, exist_ok=True)
            key = hashlib.sha256(
                bir_json if isinstance(bir_json, bytes)
                else bir_json.encode()).hexdigest()
            path = os.path.join(_NEFF_CACHE_DIR, key + ".neff")
            out_path = os.path.join(tmpdir, neff_name)
            if os.path.exists(path):
                shutil.copyfile(path, out_path)
                return out_path
            res = orig(bir_json, tmpdir, neff_name)
            shutil.copyfile(res, path)
            return res
        except Exception:
            return orig(bir_json, tmpdir, neff_name)

    cached._cached_wrapper = True
    bass_utils.compile_bir_kernel = cached
    bass2jax.compile_bir_kernel = cached


def _build(sim=False, repeat=1, repeat_full=1):
    """Build + compile the SPMD program (identical on all 8 cores).

    sim=True replaces the collective with a local DMA so the single-core
    cost-model simulator (TimelineSim) can run; timing-only, data garbage.
    repeat repeats attention+outproj inside the NEFF (timing);
    repeat_full repeats the ENTIRE body (loads, KV proj, collectives,
    attention, outproj) inside the NEFF for full-kernel timing.
    """
    exp_op, expm_op = _register_exp_op()
    nc = bacc.Bacc("TRN2", target_bir_lowering=False, debug=False,
                   num_devices=NC)
    f32, bf16 = mybir.dt.float32, mybir.dt.bfloat16
    f8 = mybir.dt.float8e4

    # host pre-reorders x/weights into SBUF order [128, CH, n] so every
    # load is a contiguous per-partition run (128 DMA descriptors, not 768)
    xq_in = nc.dram_tensor("xq", [128, CH, QC], bf16, kind="ExternalInput")
    xkv_in = nc.dram_tensor("xkv", [128, CH, QC], bf16, kind="ExternalInput")
    wq_in = nc.dram_tensor("wq", [128, CH, D], bf16, kind="ExternalInput")
    wk_in = nc.dram_tensor("wk", [128, CH, D], bf16, kind="ExternalInput")
    wv_in = nc.dram_tensor("wv", [128, CH, D], bf16, kind="ExternalInput")
    wo_in = nc.dram_tensor("wo", [128, CH, D], bf16, kind="ExternalInput")
    bq_in = nc.dram_tensor("bq", [D], f32, kind="ExternalInput")
    bk_in = nc.dram_tensor("bk", [D], f32, kind="ExternalInput")
    bv_in = nc.dram_tensor("bv", [D], f32, kind="ExternalInput")
    bo_in = nc.dram_tensor("bo", [D], f32, kind="ExternalInput")
    mk_in = nc.dram_tensor("masks", [128, 16, 128], bf16, kind="ExternalInput")
    out_ext = nc.dram_tensor("out", [QC, D], bf16, kind="ExternalOutput")

    # Two AllGathers: K fires right after K-proj (attention's scores
    # unblock ~10us earlier than a merged gather gated on V-proj), V
    # follows.  Bounce layout matches SBUF order (contiguous 6KB runs
    # per partition on both stage-out and unpack).
    KSZ = D * QC
    agin_k = nc.dram_tensor("agin_k", [KSZ], bf16)
    agout_k = nc.dram_tensor("agout_k", [NC * KSZ], bf16, addr_space="Shared")
    agin_v = nc.dram_tensor("agin_v", [KSZ], bf16)
    agout_v = nc.dram_tensor("agout_v", [NC * KSZ], bf16, addr_space="Shared")

    def kt_region(base_ap, chunk=None):
        off = 0 if chunk is None else chunk * KSZ
        return base_ap[off:off + KSZ].rearrange("(p a b) -> p a b", a=128, b=QC)

    def v_region(base_ap, chunk=None):
        off = 0 if chunk is None else chunk * KSZ
        return base_ap[off:off + KSZ].rearrange("(a s b) -> a s b", a=128, b=D)

    def w_view(src):
        return src[:, :, :]

    with tile.TileContext(nc) as tc:
      for _fr in range(repeat_full):
        FR = f"f{_fr}_"
        with (
            tc.tile_pool(name=FR + "persist", bufs=1) as persist,
            tc.tile_pool(name=FR + "stage", bufs=1) as stage,
        ):
            kt_c, v_c = [None] * NC, [None] * NC
            for i in (0, 1, 2, 3, 4, 5):
                kt_c[i] = persist.tile([128, CH, QC], bf16, tag=f"ktc{i}",
                                       name=FR + f"ktc{i}")
            for i in (0, 1):
                v_c[i] = persist.tile([128, 4, (HD + 1) * H], bf16,
                                      tag=f"vc{i}", name=FR + f"vc{i}")

            projin_cm = tc.tile_pool(name=FR + "projin", bufs=1)
            projin = projin_cm.__enter__()
            # ---- KV-critical loads first (SP + ACT queues); the K-path
            # inputs stream in two halves so the first K-proj matmuls
            # start after ~half the bytes (more pieces cost ~630ns
            # dispatch each) --------------------------------------------
            xkv_sb = projin.tile([128, CH, QC], bf16, tag="xkv")
            wk_sb = projin.tile([128, CH, D], bf16, tag="wk")
            for sl3 in (slice(0, 2), slice(2, CH)):
                nc.sync.dma_start(out=xkv_sb[:, sl3, :],
                                  in_=xkv_in[:, sl3, :])
                nc.scalar.dma_start(out=wk_sb[:, sl3, :],
                                    in_=w_view(wk_in)[:, sl3, :])
            # wv behind the K-path loads on the same queue (K first),
            # in halves so the first V-proj matmuls start earlier
            wv_sb = projin.tile([128, CH, D], bf16, tag="wv")
            for sl3 in (slice(0, 3), slice(3, CH)):
                nc.sync.dma_start(out=wv_sb[:, sl3, :],
                                  in_=w_view(wv_in)[:, sl3, :])
            bk_sb = projin.tile([128, CH], f32, tag="bk")
            nc.scalar.dma_start(
                out=bk_sb, in_=bk_in[:].rearrange("(a b) -> b a", b=128))
            bv_bc = persist.tile([128, D], f32, tag="bv")
            sap = bv_in[:]
            nc.scalar.dma_start(out=bv_bc, in_=bass.AP(
                tensor=sap.tensor, offset=sap.offset, ap=[[0, 128]] + sap.ap))

            # ---- phase A1: K/V projections -> bounce ----------------
            with tc.tile_pool(name=FR + "pj_ps", bufs=2, space="PSUM") as pj_ps:
                kt_st = stage.tile([128, CH, QC], bf16, tag="ktst")
                for p in range(CH):
                    ps = pj_ps.tile([128, QC], f32, tag="pjq")
                    cols = slice(128 * p, 128 * p + 128)
                    for d in range(CH):
                        nc.tensor.matmul(ps, wk_sb[:, d, cols], xkv_sb[:, d, :],
                                         start=(d == 0), stop=(d == CH - 1))
                    nc.vector.tensor_scalar_add(kt_st[:, p, :], ps,
                                                bk_sb[:, p:p + 1])
                nc.scalar.dma_start(out=kt_region(agin_k[:]), in_=kt_st)
                if sim:
                    # one tiny sparse DMA standing in for the collective
                    # (64B per chunk -> same dependency, no dispatch storm)
                    sap = agin_k[0:64]
                    nc.gpsimd.dma_start(
                        out=agout_k[:].rearrange(
                            "(r n) -> r n", r=NC)[:, 0:64],
                        in_=bass.AP(tensor=sap.tensor, offset=sap.offset,
                                    ap=[[0, NC]] + sap.ap))
                else:
                    nc.gpsimd.collective_compute(
                        "AllGather", mybir.AluOpType.bypass,
                        replica_groups=[list(range(NC))],
                        ins=[agin_k[:].opt()], outs=[agout_k[:].opt()])
                # first two K chunks unpack on this queue, right behind
                # the gather (both sim and real builds): they dispatch
                # the moment it completes
                for _i2 in (0, 1, 2, 3, 4, 5):
                    nc.gpsimd.dma_start(out=kt_c[_i2],
                                        in_=kt_region(agout_k[:], _i2))


                v_st = stage.tile([128, 4, D], bf16, tag="vst")
                for s4 in range(4):
                    rows = slice(128 * s4, 128 * s4 + 128)
                    psa = pj_ps.tile([128, 512], f32, tag="pjva")
                    psb = pj_ps.tile([128, 256], f32, tag="pjvb")
                    for d in range(CH):
                        lt = xkv_sb[:, d, rows]
                        nc.tensor.matmul(psa, lt, wv_sb[:, d, 0:512],
                                         start=(d == 0), stop=(d == CH - 1))
                        nc.tensor.matmul(psb, lt, wv_sb[:, d, 512:768],
                                         start=(d == 0), stop=(d == CH - 1))
                    # transpose (h,e)->(e,h) on the write so the gathered
                    # chunk unpacks contiguously; PV lhsT reads stride-12
                    va = v_st[:, s4, :].rearrange("p (e h) -> p h e", h=H)
                    nc.vector.tensor_add(va[:, 0:8, :], psa.rearrange(
                        "p (h e) -> p h e", e=HD), bv_bc[:, 0:512].rearrange(
                        "p (h e) -> p h e", e=HD))
                    nc.vector.tensor_add(va[:, 8:12, :], psb.rearrange(
                        "p (h e) -> p h e", e=HD), bv_bc[:, 512:768].rearrange(
                        "p (h e) -> p h e", e=HD))
                nc.scalar.dma_start(out=v_region(agin_v[:]), in_=v_st)

            # ---- AllGather V ----------------------------------------
            if sim:
                sap = agin_v[0:64]
                nc.gpsimd.dma_start(
                    out=agout_v[:].rearrange(
                        "(r n) -> r n", r=NC)[:, 0:64],
                    in_=bass.AP(tensor=sap.tensor, offset=sap.offset,
                                ap=[[0, NC]] + sap.ap))
            else:
                nc.gpsimd.collective_compute(
                    "AllGather", mybir.AluOpType.bypass,
                    replica_groups=[list(range(NC))],
                    ins=[agin_v[:].opt()], outs=[agout_v[:].opt()])

            # ---- Q-side loads (overlap gather); masks before wo/bo,
            # which are not needed until the output projection ----------
            mask_sb = persist.tile([128, 16, 128], bf16, tag="masks")
            nc.scalar.dma_start(out=mask_sb, in_=mk_in[:, :, :])
            xq_sb = projin.tile([128, CH, QC], bf16, tag="xq")
            nc.sync.dma_start(out=xq_sb, in_=xq_in[:, :, :])
            wq_sb = projin.tile([128, CH, D], bf16, tag="wq")
            nc.scalar.dma_start(out=wq_sb, in_=w_view(wq_in))
            bq_sb = projin.tile([128, CH], f32, tag="bq")
            nc.scalar.dma_start(
                out=bq_sb, in_=bq_in[:].rearrange("(a b) -> b a", b=128))
            # pre-warm the ACT Exp table so the first score tile doesn't
            # pay the function-set load
            warm = persist.tile([1, 2], f32, tag="actwarm")
            nc.vector.memset(warm, 0.0)
            nc.scalar.activation(out=warm, in_=warm,
                                 func=mybir.ActivationFunctionType.Exp)

            # ---- Q projection (overlaps gather) ---------------------
            with tc.tile_pool(name=FR + "pq_ps", bufs=2, space="PSUM") as pq_ps:
                qt_sb = persist.tile([128, CH, QC], bf16, tag="qt")
                for p in range(CH):
                    ps = pq_ps.tile([128, QC], f32, tag="pqq")
                    cols = slice(128 * p, 128 * p + 128)
                    for d in range(CH):
                        nc.tensor.matmul(ps, wq_sb[:, d, cols], xq_sb[:, d, :],
                                         start=(d == 0), stop=(d == CH - 1))
                    # bias add on ACT (idle in this window; the DVE chain
                    # was gating the first score matmul's PSUM reuse)
                    nc.scalar.activation(
                        out=qt_sb[:, p, :], in_=ps,
                        func=mybir.ActivationFunctionType.Identity,
                        bias=bq_sb[:, p:p + 1])

            projin_cm.__exit__(None, None, None)
            pt_cm = tc.tile_pool(name=FR + "pt_pool", bufs=4)
            pt_pool = pt_cm.__enter__()
            norm_cm = tc.tile_pool(name=FR + "norm", bufs=3)
            norm_pool = norm_cm.__enter__()

            # ---- unpack K^T and V' (ascending: matches pair 0 use) --
            # kt_c[i]: [128(=2 heads x 64), pair, 512 seq] per gather chunk
            for i in range(NC):
                if kt_c[i] is None:
                    kt_c[i] = persist.tile([128, CH, QC], bf16,
                                           tag=f"ktc{i}", name=FR + f"ktc{i}")
                if v_c[i] is None:
                    v_c[i] = persist.tile([128, 4, (HD + 1) * H], bf16,
                                          tag=f"vc{i}", name=FR + f"vc{i}")
                if i >= 6:   # 0-5 already issued right after AG-K
                    nc.sync.dma_start(out=kt_c[i],
                                      in_=kt_region(agout_k[:], i))
                nc.vector.memset(v_c[i][:, :, HD * H:], 1.0)
                nc.gpsimd.dma_start(
                    out=v_c[i][:, :, 0:HD * H],
                    in_=v_region(agout_v[:], i))

            # out-proj operands: issued after the unpacks so the DMA
            # device drains the gather stream first (wo is consumed last)
            wo_sb = persist.tile([128, CH, D], bf16, tag="wo")
            nc.scalar.dma_start(out=wo_sb, in_=w_view(wo_in))
            bo_bc = persist.tile([128, D], f32, tag="bo")
            sap = bo_in[:]
            nc.scalar.dma_start(out=bo_bc, in_=bass.AP(
                tensor=sap.tensor, offset=sap.offset, ap=[[0, 128]] + sap.ap))
            # bf16 bias row + ones row: out-proj bias rides as a final
            # 1-contraction matmul so the PSUM->SBUF move needs no DVE add
            bo_bf = persist.tile([1, D], bf16, tag="bobf")
            nc.scalar.copy(bo_bf, bo_bc[0:1, :])
            ones1 = persist.tile([1, 128], bf16, tag="ones1")
            nc.vector.memset(ones1, 1.0)

            # ---- phase B: attention (head pairs, row-group packed) --
            ot_sb = persist.tile([128, CH, QC], bf16, tag="ot")
            attn_ps = tc.tile_pool(name=FR + "attn_ps", bufs=3, space="PSUM")
            sc_ps = attn_ps.__enter__()
            pvpool = tc.tile_pool(name=FR + "pv_ps", bufs=2, space="PSUM")
            pv_ps = pvpool.__enter__()
            for _rep, pr in [(r, p) for r in range(repeat)
                             for p in (0, 3, 1, 4, 2, 5)]:
                lb = pr < 3
                rows = (slice(0, 64), slice(64, 128))
                pv2 = [pv_ps.tile([65, QC], f32, tag="pv",
                                  name=FR + f"pv{_rep}_{pr}{ab}")
                       for ab in range(2)]
                for b in range(4):      # blocks of 8 k-tiles
                    cols = slice(128 * b, QC) if lb else slice(0, QC - 128 * b)
                    pt2 = [pt_pool.tile([128, 8, QC], bf16, tag="pt",
                                        name=FR + f"pt{_rep}_{pr}{b}{ab}")
                           for ab in range(2)]
                    # narrow blocks are exp-instruction-overhead bound: use
                    # wider score groups there (same 2-bank tile footprint)
                    # wider score groups for the narrow blocks (their slot
                    # strides 1024B/512B stay PSUM-bank aligned; b0/b1 keep
                    # the [.., 2, QC] layout whose slots are exactly 1 bank)
                    kpg = {0: 2, 1: 2, 2: 4, 3: 8}[b]   # k-tiles per group
                    ng = 8 // kpg
                    ncols = QC - 128 * b
                    compact = b >= 2
                    # diagonal query strip of this block (absolute cols)
                    mq = slice(128 * b, 128 * b + 128) if lb else \
                        slice(QC - 128 * (b + 1), QC - 128 * b)
                    rest = slice(128 * b + 128, QC) if lb else \
                        slice(0, QC - 128 * (b + 1))
                    moff = 0 if lb else 8

                    def sc_cols(csl):
                        # map absolute col slice -> sc-tile col slice
                        if compact:
                            return slice(csl.start - cols.start,
                                         csl.stop - cols.start)
                        return csl

                    for gg in range(ng):
                        shape = [128, kpg, ncols] if compact else [128, kpg, QC]
                        sc2 = [sc_ps.tile(shape, f32, tag="sc",
                                          name=FR + f"sc{_rep}_{pr}_{b}_{gg}_{ab}")
                               for ab in range(2)]
                        # pt slot s is ktn-ascending for BOTH mask types
                        # (la reversed vs mm).  Scores narrowed to the
                        # group's triangular boundary (exp reads only that
                        # range, see z below).
                        zg = 16 * kpg * gg
                        gcols = (slice(cols.start + zg, cols.stop) if lb
                                 else slice(cols.start, cols.stop - zg))
                        for t in range(kpg):
                            mm = 8 * b + kpg * gg + t
                            ktn = mm if lb else KT_N - 1 - mm
                            st = t if lb else kpg - 1 - t  # sc slot
                            kk = slice(128 * (ktn % 4), 128 * (ktn % 4) + 128)
                            for ab in range(2):
                                # ab=1 runs in array rows 64-127, concurrent
                                nc.tensor.matmul(
                                    sc2[ab][:, st, sc_cols(gcols)] if compact
                                    else sc2[ab][:, st, gcols],
                                    kt_c[ktn // 4][rows[ab], pr, kk],
                                    qt_sb[rows[ab], pr, gcols],
                                    start=True, stop=True)
                        sl = (slice(kpg * gg, kpg * gg + kpg) if lb else
                              slice(8 - kpg * (gg + 1), 8 - kpg * gg))
                        mrow = slice(moff + sl.start, moff + sl.stop)
                        # triangular restriction: tiles in this group only
                        # have unmasked/partial cols in a sub-window of the
                        # strip; skip exp below/above it (pre-zeroed).
                        z = 16 * kpg * gg
                        if lb:
                            dq = slice(mq.start + z, mq.stop)       # exp'd
                            zq = slice(mq.start, mq.start + z)      # zeroed
                            mcol = slice(z, 128)
                        else:
                            dq = slice(mq.start, mq.stop - z)
                            zq = slice(mq.stop - z, mq.stop)
                            mcol = slice(0, 128 - z)
                        # no pre-zero needed: PV reads slot s only from its
                        # staircase boundary 16*s, which is >= this group's
                        # exp'd range start; below-boundary cols are never
                        # read by anything.
                        for ab in range(2):
                            if b == 3 and ab == 1:
                                # b3 has no off-diagonal work for ACT; give
                                # it this strip (exp) + mask-mul on DVE
                                nc.scalar.activation(
                                    out=pt2[ab][:, sl, dq],
                                    in_=sc2[ab][:, :, sc_cols(dq)],
                                    func=mybir.ActivationFunctionType.Exp,
                                    scale=SCALE)
                                nc.vector.tensor_mul(
                                    pt2[ab][:, sl, dq], pt2[ab][:, sl, dq],
                                    mask_sb[:, mrow, mcol])
                                continue
                            # diagonal strip: DVE poly-exp with fused mask
                            nc.vector._custom_dve(
                                expm_op,
                                out=pt2[ab][:, sl, dq],
                                in0=sc2[ab][:, :, sc_cols(dq)],
                                in1=mask_sb[:, mrow, mcol],
                                s0=_EXPC2[0], s1=_EXPC2[1])
                            # off-diagonal remainder: ACT exp (a few groups
                            # go to DVE plain poly-exp for load balance)
                            if rest.stop > rest.start:
                                if b == 0 and ab == 0 and gg < 3:
                                    # col-split between DVE and ACT so the
                                    # group's pt latency is the max of two
                                    # short passes, not DVE diag+off-diag
                                    # serial while ACT idles
                                    mid = rest.start + 256
                                    nc.vector._custom_dve(
                                        exp_op,
                                        out=pt2[ab][:, sl, rest.start:mid],
                                        in0=sc2[ab][:, :, sc_cols(
                                            slice(rest.start, mid))],
                                        s0=_EXPC[0], s1=_EXPC[1],
                                        imm2=_EXPC[2])
                                    nc.scalar.activation(
                                        out=pt2[ab][:, sl, mid:rest.stop],
                                        in_=sc2[ab][:, :, sc_cols(
                                            slice(mid, rest.stop))],
                                        func=mybir.ActivationFunctionType.Exp,
                                        scale=SCALE)
                                else:
                                    nc.scalar.activation(
                                        out=pt2[ab][:, sl, rest],
                                        in_=sc2[ab][:, :, sc_cols(rest)],
                                        func=mybir.ActivationFunctionType.Exp,
                                        scale=SCALE)
                    # PV for this block (denominator rides in row 64);
                    # pt slot s holds ktn-ascending keys for both types.
                    # Narrowed per-slot: cols below the slot's staircase
                    # boundary hold exact zeros (never contribute).
                    for s in range(8):
                        ktn = (8 * b + s if lb
                               else KT_N - 8 * (b + 1) + s)
                        scols = (slice(cols.start + 16 * s, cols.stop) if lb
                                 else slice(cols.start,
                                            cols.stop - 16 * (7 - s)))
                        vck = v_c[ktn // 4][:, ktn % 4, :].rearrange(
                            "p (e h) -> p h e", h=H)
                        for ab in range(2):
                            nc.tensor.matmul(
                                pv2[ab][:, scols], vck[:, 2 * pr + ab, :],
                                pt2[ab][:, s, scols],
                                start=(b == 0 and s == 0),
                                stop=(b == 3 and s == 7))
                # normalize: rows 0-63 / row 64, into the out-proj
                # operand.  pv2 is first copied to SBUF on ACT so its PSUM
                # bank frees quickly (otherwise the next head-pair's PV
                # stalls on the reciprocal/broadcast/mul chain).
                last = (_rep == repeat - 1 and pr == 5)
                for ab in range(2):
                    rc = norm_pool.tile([1, QC], f32, tag="rc")
                    if last:
                        # no successor needs this PSUM bank: skip the copy
                        # hop and normalize straight out of PSUM (DVE is
                        # idle by now)
                        nc.vector.reciprocal(rc, pv2[ab][64:65, :])
                        rb = norm_pool.tile([64, QC], f32, tag="rb")
                        nc.gpsimd.partition_broadcast(rb, rc)
                        nc.vector.tensor_mul(ot_sb[rows[ab], pr, :],
                                             pv2[ab][0:64, :], rb)
                        continue
                    pvs = norm_pool.tile([65, QC], f32, tag="pvs")
                    nc.scalar.copy(pvs, pv2[ab])
                    nc.vector.reciprocal(rc, pvs[64:65, :])
                    rb = norm_pool.tile([64, QC], f32, tag="rb")
                    nc.gpsimd.partition_broadcast(rb, rc)
                    # normalize mul on gpsimd (reads SBUF only) to keep the
                    # DVE free for the exp pipeline
                    nc.gpsimd.tensor_mul(ot_sb[rows[ab], pr, :],
                                         pvs[0:64, :], rb)
            pvpool.__exit__(None, None, None)
            attn_ps.__exit__(None, None, None)

            norm_cm.__exit__(None, None, None)
            pt_cm.__exit__(None, None, None)

            # ---- phase C: output projection -------------------------
            with tc.tile_pool(name=FR + "op_ps", bufs=2, space="PSUM") as op_ps:
                ob = stage.tile([128, 4, D], bf16, tag="ob")
                for qb in range(4 * repeat):
                    qb = qb % 4
                    qcols = slice(128 * qb, 128 * qb + 128)
                    psa = op_ps.tile([128, 512], f32, tag="opa")
                    psb = op_ps.tile([128, 256], f32, tag="opb")
                    for pch in range(CH):
                        lt = ot_sb[:, pch, qcols]
                        nc.tensor.matmul(psa, lt, wo_sb[:, pch, 0:512],
                                         start=(pch == 0), stop=False)
                        nc.tensor.matmul(psb, lt, wo_sb[:, pch, 512:768],
                                         start=(pch == 0), stop=False)
                    nc.tensor.matmul(psa, ones1, bo_bf[:, 0:512],
                                     start=False, stop=True)
                    nc.tensor.matmul(psb, ones1, bo_bf[:, 512:768],
                                     start=False, stop=True)
                    nc.scalar.copy(ob[:, qb, 0:512], psa)
                    nc.scalar.copy(ob[:, qb, 512:768], psb)
                    nc.sync.dma_start(
                        out=out_ext[:, :].rearrange(
                            "(q p) n -> p q n", p=128)[:, qb, :],
                        in_=ob[:, qb, :])

    nc.compile()
    return nc


def _sb_order(w):
    # [D, n] -> [128, CH, n] (SBUF order: partition-major contiguous)
    return np.ascontiguousarray(
        w.reshape(CH, 128, -1).transpose(1, 0, 2)).astype(BF16)


def _host_prep(x, Wq, bq, Wk, bk, Wv, bv, Wo, bo):
    xT = np.ascontiguousarray(
        x.reshape(S, D).T).astype(BF16)          # [768, 4096]
    wq_t = _sb_order(Wq.transpose(1, 0, 2).reshape(D, D))
    wk_t = _sb_order(Wk.transpose(1, 0, 2).reshape(D, D))
    wv_t = _sb_order(Wv.transpose(1, 0, 2).reshape(D, D))
    wo_m = _sb_order(np.asarray(Wo))
    common = {
        "wq": wq_t, "wk": wk_t, "wv": wv_t, "wo": wo_m,
        "bq": bq.reshape(D).astype(np.float32),
        "bk": bk.reshape(D).astype(np.float32),
        "bv": bv.reshape(D).astype(np.float32),
        "bo": bo.reshape(D).astype(np.float32),
    }
    k_idx = np.arange(128)[:, None]
    n_idx = np.arange(128)[None, :]
    in_maps = []
    for c in range(NC):
        # rows 0-7: lookback, tile index m (ktn-ascending).  rows 8-15:
        # lookahead, SLOT-indexed s (ktn-ascending, i.e. reversed vs the
        # former mm ordering) to match the kernel's DoubleRow slot layout.
        masks = np.zeros((128, 16, 128), dtype=BF16)
        for m in range(8):
            masks[:, m, :] = (128 * m + k_idx <= 8 * n_idx + c)
            masks[:, 8 + m, :] = (128 * m + k_idx >= 8 * n_idx + c)
        in_maps.append({
            **common,
            "xq": _sb_order(xT[:, c::NC]),
            "xkv": _sb_order(xT[:, QC * c:QC * (c + 1)]),
            "masks": masks,
        })
    return in_maps




# ---------------------------------------------------------------------------
# Cached PJRT runner: same semantics as bass2jax.run_bass_via_pjrt for the
# 8-core SPMD case, but the jitted executable is built once and reused, so
# repeat kernel() calls skip retracing (~1.6s/call -> ~transfer+exec).
_RUNNER = None


def _make_runner(nc):
    import jax
    from jax.sharding import Mesh, PartitionSpec
    from jax.experimental.shard_map import shard_map
    from concourse import bass2jax, mybir as _mb

    bass2jax.install_neuronx_cc_hook()
    partition_name = (nc.partition_id_tensor.name
                      if nc.partition_id_tensor else None)
    in_names, out_names, out_avals, zero_shapes = [], [], [], []
    for alloc in nc.m.functions[0].allocations:
        if not isinstance(alloc, _mb.MemoryLocationSet):
            continue
        name = alloc.memorylocations[0].name
        if alloc.kind == "ExternalInput":
            if name != partition_name:
                in_names.append(name)
        elif alloc.kind == "ExternalOutput":
            shape = tuple(alloc.tensor_shape)
            dtype = _mb.dt.np(alloc.dtype)
            out_names.append(name)
            out_avals.append(jax.core.ShapedArray(shape, dtype))
            zero_shapes.append((shape, dtype))
    n_params = len(in_names)
    all_names = in_names + out_names
    if partition_name is not None:
        all_names = all_names + [partition_name]
    donate = tuple(range(n_params, n_params + len(out_names)))

    def _body(*args):
        operands = list(args)
        if partition_name is not None:
            operands.append(bass2jax.partition_id_tensor())
        outs = bass2jax._bass_exec_p.bind(
            *operands,
            out_avals=tuple(out_avals),
            in_names=tuple(all_names),
            out_names=tuple(out_names),
            lowering_input_output_aliases=(),
            sim_require_finite=True,
            sim_require_nnan=True,
            nc=nc,
        )
        return tuple(outs)

    devices = jax.devices()[:NC]
    mesh = Mesh(np.asarray(devices), ("core",))
    in_specs = (PartitionSpec("core"),) * (n_params + len(out_names))
    out_specs = (PartitionSpec("core"),) * len(out_names)
    sharded = jax.jit(
        shard_map(_body, mesh=mesh, in_specs=in_specs, out_specs=out_specs,
                  check_rep=False),
        donate_argnums=donate, keep_unused=True)

    from jax.sharding import NamedSharding
    import jax.numpy as jnp
    shard = NamedSharding(mesh, PartitionSpec("core"))
    static_names = {"wq", "wk", "wv", "wo", "bq", "bk", "bv", "bo", "masks"}
    static_cache = {}

    def _zeros():
        return tuple(jnp.zeros((NC * s[0], *s[1:]), d) for s, d in zero_shapes)
    zeros_fn = jax.jit(_zeros, out_shardings=(shard,) * len(zero_shapes))

    import hashlib

    def run(in_maps):
        concat_in = []
        for nm in in_names:
            if nm in static_names:
                host = np.concatenate([np.asarray(in_maps[c][nm])
                                       for c in range(NC)], axis=0)
                key = hashlib.sha1(host.tobytes()).hexdigest()
                cached = static_cache.get(nm)
                if cached is None or cached[0] != key:
                    cached = (key, jax.device_put(host, shard))
                    static_cache[nm] = cached
                concat_in.append(cached[1])
            else:
                concat_in.append(np.concatenate(
                    [np.asarray(in_maps[c][nm]) for c in range(NC)], axis=0))
        out_arrs = sharded(*concat_in, *zeros_fn())
        return [
            {nm: np.asarray(out_arrs[i]).reshape(NC, *out_avals[i].shape)[c]
             for i, nm in enumerate(out_names)}
            for c in range(NC)
        ]

    return run


def kernel(x, Wq, bq, Wk, bk, Wv, bv, Wo, bo):
    global _BUILT, _RUNNER
    args = [np.asarray(a, dtype=np.float32)
            for a in (x, Wq, bq, Wk, bk, Wv, bv, Wo, bo)]
    if _BUILT is None:
        _install_neff_cache()
        _BUILT = _build()
        _RUNNER = _make_runner(_BUILT)
    in_maps = _host_prep(*args)
    results = _RUNNER(in_maps)
    out_full = np.empty((S, D), dtype=np.float32)
    for c in range(NC):
        out_full[c::NC] = results[c]["out"].astype(np.float32)
    return out_full.reshape(1, 16, 256, D)

